# revision 1
# baseline (speedup 1.0000x reference)
"""GaussianMixtureMLP Trainium2 kernel.

5-expert MLP mixture (128->128->128->36) over batch 65536, returning the
per-sample mixture mean and variance [65536, 18].

Strategy: data-parallel over batch across 8 NeuronCores (no collectives --
the mixture reduction is over experts, which stay core-local).

Layout: features live on SBUF partitions, batch on the free axis.  The host
pre-transposes x to [128, B] so no on-chip transposes are needed; outputs
come back as [18, B_shard] per core and are un-transposed on the host.

Per 512-column tile, per expert m:
  h1 = relu(W1[m] @ xT + b1[m])          PE matmul (float32r) + ACT relu
  h2 = relu(W2[m] @ h1 + b2[m])          PE matmul + DVE relu (load balance)
  mean_m / rawvar_m = W3[m] @ h2         PE matmuls, 18->32-padded outputs,
                                         packed 4 experts per PSUM bank via
                                         tile_position col packing
Mixture reduction: per-expert means/softplus-vars/squares are stacked in
SBUF and summed over experts with small mask matmuls on the PE (the masks
carry the 1/5 weighting).  variance = relu(E[v+m^2] - mean^2) + 1e-6.
"""

import os
import numpy as np

from concourse import bacc, bass, mybir, tile
from concourse.bass_utils import run_bass_kernel_spmd

AF = mybir.ActivationFunctionType
ALU = mybir.AluOpType
F32 = mybir.dt.float32
F32R = mybir.dt.float32r

NCORES = 8
BATCH = 65536
BSHARD = BATCH // NCORES          # 8192
TB = 512                          # free-dim tile (fp32 PSUM bank limit)
NT = BSHARD // TB                 # 16 tiles per core
NM = 5                            # experts
H = 128
O = 18

USE_F32R = os.environ.get("KERNEL_NO_F32R", "0") != "1"

_cache = {}
LAST_RESULTS = None               # test.py reads exec_time_ns off this


def _r(ap):
    return ap


def _build():
    nc = bacc.Bacc("TRN2", target_bir_lowering=False, debug=False)

    MMDT = F32R if USE_F32R else F32
    din = {}
    for name, shape, dt_ in [
        ("xT", [H, BSHARD], MMDT),
        ("w1cat", [H, NM * H], MMDT), ("w2cat", [H, NM * H], MMDT),
        ("w3m", [H, 4 * H], MMDT), ("w3v", [H, 4 * H], MMDT),
        ("w3mv4", [H, 64], MMDT),
        ("b1cat", [H, NM], F32), ("b2cat", [H, NM], F32),
        ("b3m03", [H, 1], F32), ("b3v03", [H, 1], F32), ("b3mv4", [64, 1], F32),
        ("maskA", [H, 32], MMDT), ("maskL1", [64, 32], MMDT),
        ("maskLv", [64, 32], MMDT), ("mask32", [32, 32], MMDT),
    ]:
        din[name] = nc.dram_tensor(name, shape, dt_, kind="ExternalInput").ap()
    meanT = nc.dram_tensor("meanT", [O, BSHARD], F32, kind="ExternalOutput").ap()
    varT = nc.dram_tensor("varT", [O, BSHARD], F32, kind="ExternalOutput").ap()

    with tile.TileContext(nc) as tc:
        with (
            tc.tile_pool(name="w", bufs=1) as wp,
            tc.tile_pool(name="x", bufs=3) as xp,
            tc.tile_pool(name="xs", bufs=1) as xsp,
            tc.tile_pool(name="h", bufs=2) as hp,
            tc.tile_pool(name="s", bufs=2) as sp,
            tc.tile_pool(name="o", bufs=3) as op_,
            tc.tile_pool(name="ps1", bufs=1, space="PSUM") as pp1,
            tc.tile_pool(name="ps2", bufs=2, space="PSUM") as pp2,
        ):
            w = {}
            for name in ["w1cat", "w2cat", "w3m", "w3v", "w3mv4", "b1cat",
                         "b2cat", "b3m03", "b3v03", "b3mv4", "maskA",
                         "maskL1", "maskLv", "mask32"]:
                t = wp.tile(list(din[name].shape), din[name].dtype, tag=name)
                nc.sync.dma_start(out=t, in_=din[name])
                w[name] = t

            for t in range(NT):
                xt = xp.tile([H, TB], MMDT, tag="xt")
                nc.sync.dma_start(out=xt, in_=din["xT"][:, t * TB:(t + 1) * TB])

                psA = pp1.tile([H, TB], F32, tag="yA")
                psB = pp1.tile([64, TB], F32, tag="yB")
                psC = pp1.tile([H, TB], F32, tag="yC")
                psSm = pp1.tile([32, TB], F32, tag="sm")
                psSv = pp1.tile([32, TB], F32, tag="sv")
                meanS = sp.tile([H, TB], MMDT, tag="meanS")
                varS = sp.tile([H, TB], MMDT, tag="varS")
                sqS = sp.tile([H, TB], MMDT, tag="sqS")
                Lt = sp.tile([64, TB], MMDT, tag="Lt")
                sq4 = sp.tile([32, TB], MMDT, tag="sq4")

                for m in range(NM):
                    ph1 = pp1.tile([H, TB], F32, tag="h1")
                    nc.tensor.matmul(
                        ph1, _r(w["w1cat"][:, m * H:(m + 1) * H]), _r(xt),
                        start=True, stop=True)
                    h1 = hp.tile([H, TB], MMDT, tag="h1s")
                    nc.scalar.activation(h1, ph1, AF.Relu,
                                         bias=w["b1cat"][:, m:m + 1])

                    ph2 = pp2.tile([H, TB], F32, tag="h2")
                    nc.tensor.matmul(
                        ph2, _r(w["w2cat"][:, m * H:(m + 1) * H]), _r(h1),
                        start=True, stop=True)
                    h2 = hp.tile([H, TB], MMDT, tag="h2s")
                    nc.vector.tensor_scalar(h2, ph2, w["b2cat"][:, m:m + 1],
                                            0.0, ALU.add, ALU.max)

                    if m < 4:
                        nc.tensor.matmul(psA, _r(w["w3m"][:, m * H:(m + 1) * H]),
                                         _r(h2), start=(m == 0), stop=(m == 3),
                                         skip_group_check=True)
                        nc.tensor.matmul(psC, _r(w["w3v"][:, m * H:(m + 1) * H]),
                                         _r(h2), start=(m == 0), stop=(m == 3),
                                         skip_group_check=True)
                    else:
                        nc.tensor.matmul(psB, _r(w["w3mv4"]), _r(h2),
                                         start=True, stop=True)

                # stack per-expert quantities into SBUF
                nc.scalar.activation(meanS, psA, AF.Identity, bias=w["b3m03"])
                ev = sp.tile([H, TB], F32, tag="ev")
                nc.scalar.activation(ev, psC, AF.Exp, bias=w["b3v03"])
                nc.scalar.activation(varS, ev, AF.Ln, bias=1.0)
                nc.vector.tensor_scalar(Lt[0:32, :], psB[0:32, :],
                                        w["b3mv4"][0:32, :], None, ALU.add)
                ev4 = sp.tile([64, TB], F32, tag="ev4")
                nc.scalar.activation(ev4[32:64, :], psB[32:64, :], AF.Exp,
                                     bias=w["b3mv4"][32:64, :])
                nc.scalar.activation(Lt[32:64, :], ev4[32:64, :], AF.Ln,
                                     bias=1.0)
                nc.vector.tensor_tensor(sqS, meanS, meanS, ALU.mult)
                nc.vector.tensor_tensor(sq4, Lt[0:32, :], Lt[0:32, :],
                                        ALU.mult)

                # mixture sums over experts (masks carry the 1/5)
                nc.tensor.matmul(psSm, _r(w["maskA"]), _r(meanS),
                                 start=True, stop=False)
                nc.tensor.matmul(psSm, _r(w["maskL1"]), _r(Lt),
                                 start=False, stop=True)
                nc.tensor.matmul(psSv, _r(w["maskA"]), _r(sqS),
                                 start=True, stop=False)
                nc.tensor.matmul(psSv, _r(w["mask32"]), _r(sq4),
                                 start=False, stop=False)
                nc.tensor.matmul(psSv, _r(w["maskA"]), _r(varS),
                                 start=False, stop=False)
                nc.tensor.matmul(psSv, _r(w["maskLv"]), _r(Lt),
                                 start=False, stop=True)

                # variance = relu(E[v+m^2] - mean^2) + 1e-6
                mo = op_.tile([32, TB], F32, tag="mo")
                nc.scalar.activation(mo, psSm, AF.Copy)
                mosq = op_.tile([32, TB], F32, tag="mosq")
                nc.vector.tensor_tensor(mosq, mo, mo, ALU.mult)
                vt = op_.tile([32, TB], F32, tag="vt")
                nc.vector.scalar_tensor_tensor(vt, psSv, 1e-6, mosq,
                                               ALU.add, ALU.subtract)
                vf = op_.tile([32, TB], F32, tag="vf")
                nc.vector.tensor_scalar(vf, vt, 0.0, 1e-6, ALU.max, ALU.add)

                nc.sync.dma_start(out=meanT[:, t * TB:(t + 1) * TB],
                                  in_=mo[0:O, :])
                nc.sync.dma_start(out=varT[:, t * TB:(t + 1) * TB],
                                  in_=vf[0:O, :])
    nc.compile()
    return nc


def _prep_consts(W1, b1, W2, b2, W3, b3):
    c = {}
    c["w1cat"] = np.ascontiguousarray(
        np.concatenate([W1[m].T for m in range(NM)], axis=1), np.float32)
    c["w2cat"] = np.ascontiguousarray(
        np.concatenate([W2[m].T for m in range(NM)], axis=1), np.float32)
    w3m = np.zeros((H, 4 * H), np.float32)
    w3v = np.zeros((H, 4 * H), np.float32)
    for m in range(4):
        w3m[:, m * H + m * 32:m * H + m * 32 + O] = W3[m, 0:O, :].T
        w3v[:, m * H + m * 32:m * H + m * 32 + O] = W3[m, O:2 * O, :].T
    c["w3m"], c["w3v"] = w3m, w3v
    w3mv4 = np.zeros((H, 64), np.float32)
    w3mv4[:, 0:O] = W3[4, 0:O, :].T
    w3mv4[:, 32:32 + O] = W3[4, O:2 * O, :].T
    c["w3mv4"] = w3mv4
    c["b1cat"] = np.ascontiguousarray(b1.T, np.float32)
    c["b2cat"] = np.ascontiguousarray(b2.T, np.float32)
    b3m03 = np.zeros((H, 1), np.float32)
    b3v03 = np.zeros((H, 1), np.float32)
    for m in range(4):
        b3m03[m * 32:m * 32 + O, 0] = b3[m, 0:O]
        b3v03[m * 32:m * 32 + O, 0] = b3[m, O:2 * O]
    c["b3m03"], c["b3v03"] = b3m03, b3v03
    b3mv4 = np.zeros((64, 1), np.float32)
    b3mv4[0:O, 0] = b3[4, 0:O]
    b3mv4[32:32 + O, 0] = b3[4, O:2 * O]
    c["b3mv4"] = b3mv4
    maskA = np.zeros((H, 32), np.float32)
    for m in range(4):
        for r in range(O):
            maskA[m * 32 + r, r] = 0.2
    c["maskA"] = maskA
    maskL1 = np.zeros((64, 32), np.float32)
    maskLv = np.zeros((64, 32), np.float32)
    mask32 = np.zeros((32, 32), np.float32)
    for r in range(O):
        maskL1[r, r] = 0.2
        maskLv[32 + r, r] = 0.2
        mask32[r, r] = 0.2
    c["maskL1"], c["maskLv"], c["mask32"] = maskL1, maskLv, mask32
    return c


def kernel(x, W1, b1, W2, b2, W3, b3):
    global LAST_RESULTS
    if "nc" not in _cache:
        _cache["nc"] = _build()
    nc = _cache["nc"]

    consts = _prep_consts(np.asarray(W1), np.asarray(b1), np.asarray(W2),
                          np.asarray(b2), np.asarray(W3), np.asarray(b3))
    xT = np.ascontiguousarray(np.asarray(x).T, np.float32)  # [128, B]

    in_maps = []
    for cix in range(NCORES):
        m = dict(consts)
        m["xT"] = np.ascontiguousarray(xT[:, cix * BSHARD:(cix + 1) * BSHARD])
        in_maps.append(m)

    trace = os.environ.get("KERNEL_TRACE", "0") == "1"
    res = run_bass_kernel_spmd(nc, in_maps, list(range(NCORES)), trace=trace)
    LAST_RESULTS = res

    mean = np.concatenate([r["meanT"] for r in res.results], axis=1).T
    var = np.concatenate([r["varT"] for r in res.results], axis=1).T
    return (np.ascontiguousarray(mean), np.ascontiguousarray(var))



# revision 16
# speedup vs baseline: 1.3418x; 1.3418x over previous
"""GaussianMixtureMLP Trainium2 kernel (v3.2).

5-expert MLP mixture (128->128->128->2*18) over batch 65536, returning the
per-sample mixture mean and variance [65536, 18].

Data-parallel over batch across 8 NeuronCores (no collectives -- the
mixture reduction is over experts, which stay core-local).

Per 512-column tile (16 tiles/core), per expert m:
  h1 = relu(W1[m] @ x + b1)   PE matmul + ACT/DVE relu (bf16 out)
  h2 = relu(W2[m] @ h1 + b2)  PE matmul + ACT/DVE relu
  W3 combined:                ONE matmul per expert into packed PSUM.

PSUM packing (all ALU reads start at 32-aligned partitions; junk rows in
the over-read windows are zeroed by the mask matmuls downstream):
  psA [128,512] (experts 0-2): means 0:54 | 0.2*mean mix-acc 54:72 |
                               vars 72:126 | zeros
  psB [128,512] (experts 3-4): means 0:36 | zeros | mix-acc 54:72 |
                               vars 72:108 | zeros
Post-processing per tile (softplus = ln(1+exp), no softplus table):
  sqX   = ACT Square(psX[0:64] + b3mean)        -> M[0:64]
  spX   = ACT Exp(psX[64:128] + b3var) ACT Ln   -> M[64:128]
  mean  = DVE stt(psA[32:96] + bmix + psB[32:96]) -> mean4 64-row block
          (true rows at block offset 22:40, matching the mask layout)
  2 mask matmuls (0.2 baked in) -> psSv4 64-row block, cols 22:40
Finals per 2-tile group as [128,512] ops (free-dim-only engine cost):
  m2g = mean4^2 (DVE bf16 2x), vg = psSv4+1e-6-m2g (DVE),
  vfg = relu(vg)+1e-6 (Pool).  DMA out rows 22:40 / 86:104.
"""

import os
import numpy as np
import ml_dtypes

from concourse import bacc, bass, mybir, tile
from concourse.bass_utils import run_bass_kernel_spmd

AF = mybir.ActivationFunctionType
ALU = mybir.AluOpType
F32 = mybir.dt.float32
BF16 = mybir.dt.bfloat16

NCORES = 8
BATCH = 65536
BSHARD = BATCH // NCORES          # 8192
TB = 512                          # free-dim tile (fp32 PSUM bank limit)
NT = BSHARD // TB                 # 16 tiles per core
GR = 3                            # tiles per group (32-row blocks at 0/32/64)
NM = 5
H = 128
O = 18
ROFF = 0                          # true-row offset inside each 32-row block

_cache = {}
LAST_RESULTS = None


def _build():
    nc = bacc.Bacc("TRN2", target_bir_lowering=False, debug=False)

    din = {}
    for name, shape, dt_ in [
        ("xT", [H, BSHARD], BF16),
        ("w1cat", [H, NM * H], BF16), ("w2cat", [H, NM * H], BF16),
        ("w3A", [H, 3 * H], BF16), ("w3B", [H, 2 * H], BF16),
        ("w3mixB", [H, 64], BF16),
        ("mkSq", [H, 32], BF16), ("mkSpA", [H, 32], BF16),
        ("mkSpB", [H, 32], BF16),
        ("b1cat", [H, NM], F32), ("b2cat", [H, NM], F32),
        ("bA64", [64, 1], F32), ("bvA128", [H, 1], F32),
        ("bB64", [64, 1], F32), ("bvB128", [H, 1], F32),
        ("bmix32", [32, 1], F32),
    ]:
        din[name] = nc.dram_tensor(name, shape, dt_, kind="ExternalInput").ap()
    meanT = nc.dram_tensor("meanT", [O, BSHARD], BF16, kind="ExternalOutput").ap()
    varT = nc.dram_tensor("varT", [O, BSHARD], BF16, kind="ExternalOutput").ap()

    with tile.TileContext(nc) as tc:
        with (
            tc.tile_pool(name="w", bufs=1) as wp,
            tc.tile_pool(name="x", bufs=3) as xp,
            tc.tile_pool(name="h", bufs=2) as hp,
            tc.tile_pool(name="m", bufs=2) as mp,
            tc.tile_pool(name="g", bufs=2) as gp,
            tc.tile_pool(name="ps1", bufs=2, space="PSUM") as ph1,
            tc.tile_pool(name="ps2", bufs=3, space="PSUM") as ph2,
            tc.tile_pool(name="psa", bufs=1, space="PSUM") as pA,
            tc.tile_pool(name="psb", bufs=1, space="PSUM") as pB,
            tc.tile_pool(name="pss", bufs=1, space="PSUM") as pS,
        ):
            w = {}
            for name in ["w1cat", "w2cat", "w3A", "w3B", "w3mixB", "mkSq",
                         "mkSpA", "mkSpB", "b1cat", "b2cat", "bA64", "bvA128",
                         "bB64", "bvB128", "bmix32"]:
                t = wp.tile(list(din[name].shape), din[name].dtype, tag=name)
                nc.sync.dma_start(out=t, in_=din[name])
                w[name] = t

            def xdma(t):
                xt = xp.tile([H, TB], BF16, tag="xt")
                nc.sync.dma_start(out=xt, in_=din["xT"][:, t * TB:(t + 1) * TB])
                return xt

            xts = {0: xdma(0), 1: xdma(1)}

            psSv4 = None
            mean4 = None
            for t in range(NT):
                g, k = t // GR, t % GR
                if t + 2 < NT:
                    xts[t + 2] = xdma(t + 2)
                xt = xts.pop(t)

                gsize = min(GR, NT - g * GR)
                rows = 32 * gsize
                if k == 0:
                    psSv4 = pS.tile([rows, TB], F32, tag="psSv4")
                    mean4 = gp.tile([rows, TB], BF16, tag="mean4")

                psA = pA.tile([H, TB], F32, tag="psA")
                psB = pB.tile([H, TB], F32, tag="psB")

                for m in range(NM):
                    phA = ph1.tile([H, TB], F32, tag="psH1")
                    nc.tensor.matmul(phA, w["w1cat"][:, m * H:(m + 1) * H],
                                     xt, start=True, stop=True)
                    h1 = hp.tile([H, TB], BF16, tag="h1")
                    if m < 2:
                        nc.scalar.activation(h1, phA, AF.Relu,
                                             bias=w["b1cat"][:, m:m + 1])
                    else:
                        nc.vector.tensor_scalar(h1, phA, w["b1cat"][:, m:m + 1],
                                                0.0, ALU.add, ALU.max)

                    phB = ph2.tile([H, TB], F32, tag="psH2")
                    nc.tensor.matmul(phB, w["w2cat"][:, m * H:(m + 1) * H],
                                     h1, start=True, stop=True)
                    h2 = hp.tile([H, TB], BF16, tag="h2")
                    if m < 2:
                        nc.scalar.activation(h2, phB, AF.Relu,
                                             bias=w["b2cat"][:, m:m + 1])
                    else:
                        nc.vector.tensor_scalar(h2, phB, w["b2cat"][:, m:m + 1],
                                                0.0, ALU.add, ALU.max)

                    if m < 3:
                        nc.tensor.matmul(psA, w["w3A"][:, m * H:(m + 1) * H],
                                         h2, start=(m == 0), stop=False,
                                         skip_group_check=True)
                    else:
                        nc.tensor.matmul(psB, w["w3B"][:, (m - 3) * H:(m - 2) * H],
                                         h2, start=(m == 3), stop=(m == 4),
                                         skip_group_check=True)
                        # expert 3/4 mixture-mean contribution into psA's
                        # mix rows (psA is the single-PSUM mean source)
                        nc.tensor.matmul(psA[64:96, :],
                                         w["w3mixB"][:, (m - 3) * 32:(m - 2) * 32],
                                         h2, start=False, stop=(m == 4),
                                         skip_group_check=True)

                M23 = mp.tile([H, TB], BF16, tag="M23")
                L2 = mp.tile([H, TB], BF16, tag="L2")
                L3 = mp.tile([H, TB], BF16, tag="L3")
                EA = mp.tile([H, TB], BF16, tag="EA")
                EB = mp.tile([H, TB], BF16, tag="EB")
                # squares from the [0:64] windows (A and B packed into one
                # tile -> one mask pass); softplus from full-tile exp/ln
                # (junk rows cost nothing and are masked out below)
                nc.scalar.activation(M23[0:64, :], psA[0:64, :], AF.Square,
                                     bias=w["bA64"])
                nc.scalar.activation(M23[64:128, :], psB[0:64, :], AF.Square,
                                     bias=w["bB64"])
                nc.scalar.activation(EA, psA, AF.Exp, bias=w["bvA128"])
                nc.scalar.activation(L2, EA, AF.Ln, bias=1.0)
                nc.scalar.activation(EB, psB, AF.Exp, bias=w["bvB128"])
                nc.scalar.activation(L3, EB, AF.Ln, bias=1.0)

                nc.vector.tensor_scalar(mean4[32 * k:32 * (k + 1), :],
                                        psA[64:96, :], w["bmix32"],
                                        None, ALU.add)

                blk = psSv4[32 * k:32 * (k + 1), :]
                nc.tensor.matmul(blk, w["mkSq"], M23, start=True, stop=False,
                                 skip_group_check=True)
                nc.tensor.matmul(blk, w["mkSpA"], L2, start=False, stop=False,
                                 skip_group_check=True)
                nc.tensor.matmul(blk, w["mkSpB"], L3, start=False, stop=True,
                                 skip_group_check=True)

                nc.sync.dma_start(
                    out=meanT[:, t * TB:(t + 1) * TB],
                    in_=mean4[32 * k + ROFF:32 * k + ROFF + O, :])

                if k == gsize - 1:
                    m2g = gp.tile([rows, TB], BF16, tag="m2g")
                    nc.gpsimd.tensor_tensor(m2g, mean4, mean4, ALU.mult)
                    vg = gp.tile([rows, TB], F32, tag="vg")
                    nc.vector.scalar_tensor_tensor(vg, psSv4, 1e-6, m2g,
                                                   ALU.add, ALU.subtract)
                    vfg = gp.tile([rows, TB], BF16, tag="vfg")
                    nc.gpsimd.tensor_scalar(vfg, vg, 0.0, 1e-6,
                                            ALU.max, ALU.add)
                    for kk in range(gsize):
                        tt = g * GR + kk
                        nc.sync.dma_start(
                            out=varT[:, tt * TB:(tt + 1) * TB],
                            in_=vfg[32 * kk + ROFF:32 * kk + ROFF + O, :])
    nc.compile()
    return nc


def _prep_consts(W1, b1, W2, b2, W3, b3):
    bf = ml_dtypes.bfloat16
    c = {}
    c["w1cat"] = np.concatenate([W1[m].T for m in range(NM)], axis=1).astype(bf)
    c["w2cat"] = np.concatenate([W2[m].T for m in range(NM)], axis=1).astype(bf)

    # psA rows: means 0:54 | varsA slots 54:64 and 82:126 (scattered
    # around the mix window -- the full-tile exp doesn't care, the masks
    # select by row) | mix accumulator 64:82.  psB rows: means 0:36,
    # varsB 54:90.  Experts 3/4 add their mix part via [H,32] matmuls
    # into psA[64:96].
    def va_row(v):
        return 54 + v if v < 10 else 72 + v

    def w3blk(m):
        blk = np.zeros((H, H), np.float32)
        if m < 3:
            blk[:, m * O:(m + 1) * O] = W3[m, 0:O, :].T
            for i in range(O):
                blk[:, va_row(m * O + i)] = W3[m, O + i, :].T
            blk[:, 64:82] = 0.2 * W3[m, 0:O, :].T
        else:
            j = m - 3
            blk[:, j * O:(j + 1) * O] = W3[m, 0:O, :].T
            blk[:, 54 + j * O:54 + (j + 1) * O] = W3[m, O:2 * O, :].T
        return blk

    c["w3A"] = np.concatenate([w3blk(m) for m in range(3)], axis=1).astype(bf)
    c["w3B"] = np.concatenate([w3blk(m) for m in (3, 4)], axis=1).astype(bf)
    w3mixB = np.zeros((H, 64), np.float32)
    for j, m in enumerate((3, 4)):
        w3mixB[:, 32 * j:32 * j + O] = 0.2 * W3[m, 0:O, :].T
    c["w3mixB"] = w3mixB.astype(bf)

    c["b1cat"] = np.ascontiguousarray(b1.T, np.float32)
    c["b2cat"] = np.ascontiguousarray(b2.T, np.float32)

    bA64 = np.zeros((64, 1), np.float32)
    bA64[0:54, 0] = np.concatenate([b3[m, 0:O] for m in range(3)])
    bvA128 = np.zeros((H, 1), np.float32)
    bvarA = np.concatenate([b3[m, O:2 * O] for m in range(3)])
    for v in range(54):
        bvA128[va_row(v), 0] = bvarA[v]
    bB64 = np.zeros((64, 1), np.float32)
    bB64[0:36, 0] = np.concatenate([b3[m, 0:O] for m in (3, 4)])
    bvB128 = np.zeros((H, 1), np.float32)
    bvB128[54:90, 0] = np.concatenate([b3[m, O:2 * O] for m in (3, 4)])
    bmix32 = np.zeros((32, 1), np.float32)
    bmix32[ROFF:ROFF + O, 0] = 0.2 * b3[:, 0:O].sum(axis=0)
    c["bA64"], c["bvA128"] = bA64, bvA128
    c["bB64"], c["bvB128"] = bB64, bvB128
    c["bmix32"] = bmix32

    # mixture masks: select the true sq/sp rows, write output col
    # ROFF+(r%18), 0.2 mixture weight baked in.  mkSq covers the packed
    # squares tile M23 (sqA rows 0:54, sqB rows 64:100).
    mkSq = np.zeros((H, 32), np.float32)
    for r in range(54):
        mkSq[r, r % O] = 0.2
    for r in range(64, 100):
        mkSq[r, (r - 64) % O] = 0.2
    mkSpA = np.zeros((H, 32), np.float32)
    for v in range(54):
        mkSpA[va_row(v), v % O] = 0.2
    mkSpB = np.zeros((H, 32), np.float32)
    for r in range(54, 90):
        mkSpB[r, (r - 54) % O] = 0.2
    c["mkSq"] = mkSq.astype(bf)
    c["mkSpA"] = mkSpA.astype(bf)
    c["mkSpB"] = mkSpB.astype(bf)
    return c


def kernel(x, W1, b1, W2, b2, W3, b3):
    global LAST_RESULTS
    if "nc" not in _cache:
        _cache["nc"] = _build()
    nc = _cache["nc"]

    consts = _prep_consts(np.asarray(W1), np.asarray(b1), np.asarray(W2),
                          np.asarray(b2), np.asarray(W3), np.asarray(b3))
    xT = np.asarray(x).T.astype(ml_dtypes.bfloat16)  # [128, B]

    in_maps = []
    for cix in range(NCORES):
        m = dict(consts)
        m["xT"] = np.ascontiguousarray(xT[:, cix * BSHARD:(cix + 1) * BSHARD])
        in_maps.append(m)

    trace = os.environ.get("KERNEL_TRACE", "0") == "1"
    res = run_bass_kernel_spmd(nc, in_maps, list(range(NCORES)), trace=trace)
    LAST_RESULTS = res

    mean = np.concatenate(
        [r["meanT"].astype(np.float32) for r in res.results], axis=1).T
    var = np.concatenate(
        [r["varT"].astype(np.float32) for r in res.results], axis=1).T
    return (np.ascontiguousarray(mean), np.ascontiguousarray(var))


# revision 24
# speedup vs baseline: 1.5777x; 1.1759x over previous
"""GaussianMixtureMLP Trainium2 kernel (v3.2).

5-expert MLP mixture (128->128->128->2*18) over batch 65536, returning the
per-sample mixture mean and variance [65536, 18].

Data-parallel over batch across 8 NeuronCores (no collectives -- the
mixture reduction is over experts, which stay core-local).

Per 512-column tile (16 tiles/core), per expert m:
  h1 = relu(W1[m] @ x + b1)   PE matmul + ACT/DVE relu (bf16 out)
  h2 = relu(W2[m] @ h1 + b2)  PE matmul + ACT/DVE relu
  W3 combined:                ONE matmul per expert into packed PSUM.

PSUM packing (all ALU reads start at 32-aligned partitions; junk rows in
the over-read windows are zeroed by the mask matmuls downstream):
  psA [128,512] (experts 0-2): means 0:54 | 0.2*mean mix-acc 54:72 |
                               vars 72:126 | zeros
  psB [128,512] (experts 3-4): means 0:36 | zeros | mix-acc 54:72 |
                               vars 72:108 | zeros
Post-processing per tile (softplus = ln(1+exp), no softplus table):
  sqX   = ACT Square(psX[0:64] + b3mean)        -> M[0:64]
  spX   = ACT Exp(psX[64:128] + b3var) ACT Ln   -> M[64:128]
  mean  = DVE stt(psA[32:96] + bmix + psB[32:96]) -> mean4 64-row block
          (true rows at block offset 22:40, matching the mask layout)
  2 mask matmuls (0.2 baked in) -> psSv4 64-row block, cols 22:40
Finals per 2-tile group as [128,512] ops (free-dim-only engine cost):
  m2g = mean4^2 (DVE bf16 2x), vg = psSv4+1e-6-m2g (DVE),
  vfg = relu(vg)+1e-6 (Pool).  DMA out rows 22:40 / 86:104.
"""

import os
import numpy as np
import ml_dtypes

from concourse import bacc, bass, mybir, tile
from concourse.bass_utils import run_bass_kernel_spmd

AF = mybir.ActivationFunctionType
ALU = mybir.AluOpType
F32 = mybir.dt.float32
BF16 = mybir.dt.bfloat16

NCORES = 8
BATCH = 65536
BSHARD = BATCH // NCORES          # 8192
TB = 512                          # free-dim tile (fp32 PSUM bank limit)
NT = BSHARD // TB                 # 16 tiles per core
GR = 3                            # tiles per group (32-row blocks at 0/32/64)
NM = 5
H = 128
O = 18
ROFF = 0                          # true-row offset inside each 32-row block

_cache = {}
LAST_RESULTS = None


def _build():
    nc = bacc.Bacc("TRN2", target_bir_lowering=False, debug=False)

    # all weights travel as two packed tensors (two DMAs at startup
    # instead of thirteen serialized ones)
    W16 = {"w1cat": NM * H, "w2cat": NM * H, "w3A": 3 * H, "w3B": 2 * H,
           "w3mixB": 64, "mkSq": 32, "mkSpA": 32, "mkSpB": 32}
    WF32 = {"b1cat": NM, "b2cat": NM, "bA64": 1, "bvA128": 1, "bB64": 1,
            "bvB128": 1, "bmix32": 1}
    din = {}
    for name, shape, dt_ in [
        ("xT", [H, BSHARD], BF16),
        ("wpack16", [H, sum(W16.values())], BF16),
        ("wpackf", [H, sum(WF32.values())], F32),
    ]:
        din[name] = nc.dram_tensor(name, shape, dt_, kind="ExternalInput").ap()
    meanT = nc.dram_tensor("meanT", [O, BSHARD], BF16, kind="ExternalOutput").ap()
    varT = nc.dram_tensor("varT", [O, BSHARD], BF16, kind="ExternalOutput").ap()

    with tile.TileContext(nc) as tc:
        with (
            tc.tile_pool(name="w", bufs=1) as wp,
            tc.tile_pool(name="x", bufs=3) as xp,
            tc.tile_pool(name="h", bufs=6) as hp,
            tc.tile_pool(name="m", bufs=4) as mp,
            tc.tile_pool(name="g", bufs=2) as gp,
            tc.tile_pool(name="e", bufs=6) as ep,
            tc.tile_pool(name="v", bufs=6) as vp,
            tc.tile_pool(name="l", bufs=2) as lp,
            tc.tile_pool(name="ps1", bufs=2, space="PSUM") as ph1,
            tc.tile_pool(name="ps2", bufs=3, space="PSUM") as ph2,
            tc.tile_pool(name="psa", bufs=1, space="PSUM") as pA,
            tc.tile_pool(name="psb", bufs=1, space="PSUM") as pB,
            tc.tile_pool(name="pss", bufs=1, space="PSUM") as pS,
        ):
            def xdma(t):
                xt = xp.tile([H, TB], BF16, tag="xt")
                nc.sync.dma_start(out=xt, in_=din["xT"][:, t * TB:(t + 1) * TB])
                return xt

            xts = {0: xdma(0)}
            w16 = wp.tile([H, sum(W16.values())], BF16, tag="w16")
            nc.sync.dma_start(out=w16, in_=din["wpack16"])
            wf = wp.tile([H, sum(WF32.values())], F32, tag="wf")
            nc.sync.dma_start(out=wf, in_=din["wpackf"])
            xts[1] = xdma(1)

            w = {}
            off = 0
            for name, width in W16.items():
                w[name] = w16[:, off:off + width]
                off += width
            off = 0
            for name, width in WF32.items():
                w[name] = wf[:, off:off + width]
                off += width
            w["bA64"] = w["bA64"][0:64, :]
            w["bB64"] = w["bB64"][0:64, :]
            w["bmix32"] = w["bmix32"][0:32, :]

            psSv4 = None
            mean4 = None
            EABg = None
            gVG1 = []
            gEAB = []
            for t in range(NT):
                g, k = t // GR, t % GR
                if t + 2 < NT:
                    xts[t + 2] = xdma(t + 2)
                xt = xts.pop(t)

                gsize = min(GR, NT - g * GR)
                rows = 32 * gsize
                if k == 0:
                    psSv4 = pS.tile([rows, TB], F32, tag="psSv4")
                    mean4 = gp.tile([rows, TB], BF16, tag="mean4")
                    EABg = ep.tile([H, 2 * gsize * TB], BF16, tag="EAB")

                psA = pA.tile([H, TB], F32, tag="psA")
                psB = pB.tile([H, TB], F32, tag="psB")

                # software-pipelined emission: W2(m) trails W1(m+2) and
                # W3(m) trails further, so parked matmuls (waiting on a
                # relu) never head-of-line-block the PE queue.
                h1s, h2s = {}, {}

                def emit_w1(m):
                    phA = ph1.tile([H, TB], F32, tag="psH1")
                    nc.tensor.matmul(phA, w["w1cat"][:, m * H:(m + 1) * H],
                                     xt, start=True, stop=True)
                    h1 = hp.tile([H, TB], BF16, tag="h1")
                    if m < 2:
                        nc.scalar.activation(h1, phA, AF.Relu,
                                             bias=w["b1cat"][:, m:m + 1])
                    else:
                        nc.vector.tensor_scalar(h1, phA, w["b1cat"][:, m:m + 1],
                                                0.0, ALU.add, ALU.max)
                    h1s[m] = h1

                def emit_w2(m):
                    phB = ph2.tile([H, TB], F32, tag="psH2")
                    nc.tensor.matmul(phB, w["w2cat"][:, m * H:(m + 1) * H],
                                     h1s[m], start=True, stop=True)
                    h2 = hp.tile([H, TB], BF16, tag="h2")
                    if m == 0:
                        nc.scalar.activation(h2, phB, AF.Relu,
                                             bias=w["b2cat"][:, m:m + 1])
                    else:
                        nc.vector.tensor_scalar(h2, phB, w["b2cat"][:, m:m + 1],
                                                0.0, ALU.add, ALU.max)
                    h2s[m] = h2

                def emit_w3(m):
                    h2 = h2s[m]
                    if m < 3:
                        nc.tensor.matmul(psA, w["w3A"][:, m * H:(m + 1) * H],
                                         h2, start=(m == 0), stop=False,
                                         skip_group_check=True)
                    else:
                        nc.tensor.matmul(psB, w["w3B"][:, (m - 3) * H:(m - 2) * H],
                                         h2, start=(m == 3), stop=(m == 4),
                                         skip_group_check=True)
                        # expert 3/4 mixture-mean contribution into psA's
                        # mix rows (psA is the single-PSUM mean source)
                        nc.tensor.matmul(psA[64:96, :],
                                         w["w3mixB"][:, (m - 3) * 32:(m - 2) * 32],
                                         h2, start=False, stop=(m == 4),
                                         skip_group_check=True)

                for m in range(NM):
                    emit_w1(m)
                    if m >= 2:
                        emit_w2(m - 2)
                    if m >= 4:
                        emit_w3(m - 4)
                for m in range(3, NM):
                    emit_w2(m)
                    emit_w3(m - 3)
                for m in range(2, NM):
                    emit_w3(m)

                M23 = mp.tile([H, TB], BF16, tag="M23")
                # squares from the [0:64] windows (A and B packed into one
                # tile -> one mask pass, applied in-loop); exps land in the
                # group-wide EAB tile, their ln (the only ACT function on a
                # different activation table) runs in the post-loop tail so
                # the table is loaded twice per kernel, not per tile.
                nc.scalar.activation(M23[0:64, :], psA[0:64, :], AF.Square,
                                     bias=w["bA64"])
                nc.scalar.activation(M23[64:128, :], psB[0:64, :], AF.Square,
                                     bias=w["bB64"])
                nc.scalar.activation(EABg[:, k * TB:(k + 1) * TB], psA,
                                     AF.Exp, bias=w["bvA128"])
                nc.scalar.activation(
                    EABg[:, (gsize + k) * TB:(gsize + k + 1) * TB], psB,
                    AF.Exp, bias=w["bvB128"])

                nc.tensor.matmul(psSv4[32 * k:32 * (k + 1), :], w["mkSq"],
                                 M23, start=True, stop=True,
                                 skip_group_check=True)

                nc.vector.tensor_scalar(mean4[32 * k:32 * (k + 1), :],
                                        psA[64:96, :], w["bmix32"],
                                        None, ALU.add)

                nc.sync.dma_start(
                    out=meanT[:, t * TB:(t + 1) * TB],
                    in_=mean4[32 * k + ROFF:32 * k + ROFF + O, :])

                if k == gsize - 1:
                    m2g = gp.tile([rows, TB], BF16, tag="m2g")
                    nc.gpsimd.tensor_tensor(m2g, mean4, mean4, ALU.mult)
                    vg1 = vp.tile([rows, TB], F32, tag="vg1")
                    nc.vector.scalar_tensor_tensor(vg1, psSv4, 1e-6, m2g,
                                                   ALU.add, ALU.subtract)
                    gVG1.append(vg1)
                    gEAB.append((EABg, gsize))

            # ---- tail: softplus ln + sp mixture sums + variance finals
            # (negative-priority so the scheduler keeps every ln after the
            # main loop's exp/relu/square ACT ops: one ln-table load) ----
            ctx_tail = tc.high_priority(offset=-10**6)
            ctx_tail.__enter__()
            for g, ((EABt, gsize), vg1) in enumerate(zip(gEAB, gVG1)):
                rows = 32 * gsize
                Lg = lp.tile([H, 2 * gsize * TB], BF16, tag="Lg")
                nc.scalar.activation(Lg, EABt, AF.Ln, bias=1.0)
                psSvT = pS.tile([rows, TB], F32, tag="psSv4")
                for kk in range(gsize):
                    blk = psSvT[32 * kk:32 * (kk + 1), :]
                    nc.tensor.matmul(blk, w["mkSpA"],
                                     Lg[:, kk * TB:(kk + 1) * TB],
                                     start=True, stop=False,
                                     skip_group_check=True)
                    nc.tensor.matmul(
                        blk, w["mkSpB"],
                        Lg[:, (gsize + kk) * TB:(gsize + kk + 1) * TB],
                        start=False, stop=True, skip_group_check=True)
                vgf = gp.tile([rows, TB], F32, tag="vgf")
                nc.vector.tensor_tensor(vgf, psSvT, vg1, ALU.add)
                vfg = gp.tile([rows, TB], BF16, tag="vfg")
                nc.gpsimd.tensor_scalar(vfg, vgf, 0.0, 1e-6,
                                        ALU.max, ALU.add)
                for kk in range(gsize):
                    tt = g * GR + kk
                    nc.sync.dma_start(
                        out=varT[:, tt * TB:(tt + 1) * TB],
                        in_=vfg[32 * kk + ROFF:32 * kk + ROFF + O, :])
            ctx_tail.__exit__(None, None, None)
    nc.compile()
    return nc


def _prep_consts(W1, b1, W2, b2, W3, b3):
    bf = ml_dtypes.bfloat16
    c = {}
    PACK16 = ["w1cat", "w2cat", "w3A", "w3B", "w3mixB", "mkSq", "mkSpA",
              "mkSpB"]
    PACKF = ["b1cat", "b2cat", "bA64", "bvA128", "bB64", "bvB128", "bmix32"]
    c["w1cat"] = np.concatenate([W1[m].T for m in range(NM)], axis=1).astype(bf)
    c["w2cat"] = np.concatenate([W2[m].T for m in range(NM)], axis=1).astype(bf)

    # psA rows: means 0:54 | varsA slots 54:64 and 82:126 (scattered
    # around the mix window -- the full-tile exp doesn't care, the masks
    # select by row) | mix accumulator 64:82.  psB rows: means 0:36,
    # varsB 54:90.  Experts 3/4 add their mix part via [H,32] matmuls
    # into psA[64:96].
    def va_row(v):
        return 54 + v if v < 10 else 72 + v

    def w3blk(m):
        blk = np.zeros((H, H), np.float32)
        if m < 3:
            blk[:, m * O:(m + 1) * O] = W3[m, 0:O, :].T
            for i in range(O):
                blk[:, va_row(m * O + i)] = W3[m, O + i, :].T
            blk[:, 64:82] = 0.2 * W3[m, 0:O, :].T
        else:
            j = m - 3
            blk[:, j * O:(j + 1) * O] = W3[m, 0:O, :].T
            blk[:, 54 + j * O:54 + (j + 1) * O] = W3[m, O:2 * O, :].T
        return blk

    c["w3A"] = np.concatenate([w3blk(m) for m in range(3)], axis=1).astype(bf)
    c["w3B"] = np.concatenate([w3blk(m) for m in (3, 4)], axis=1).astype(bf)
    w3mixB = np.zeros((H, 64), np.float32)
    for j, m in enumerate((3, 4)):
        w3mixB[:, 32 * j:32 * j + O] = 0.2 * W3[m, 0:O, :].T
    c["w3mixB"] = w3mixB.astype(bf)

    c["b1cat"] = np.ascontiguousarray(b1.T, np.float32)
    c["b2cat"] = np.ascontiguousarray(b2.T, np.float32)

    bA64 = np.zeros((64, 1), np.float32)
    bA64[0:54, 0] = np.concatenate([b3[m, 0:O] for m in range(3)])
    bvA128 = np.zeros((H, 1), np.float32)
    bvarA = np.concatenate([b3[m, O:2 * O] for m in range(3)])
    for v in range(54):
        bvA128[va_row(v), 0] = bvarA[v]
    bB64 = np.zeros((64, 1), np.float32)
    bB64[0:36, 0] = np.concatenate([b3[m, 0:O] for m in (3, 4)])
    bvB128 = np.zeros((H, 1), np.float32)
    bvB128[54:90, 0] = np.concatenate([b3[m, O:2 * O] for m in (3, 4)])
    bmix32 = np.zeros((32, 1), np.float32)
    bmix32[ROFF:ROFF + O, 0] = 0.2 * b3[:, 0:O].sum(axis=0)
    c["bA64"], c["bvA128"] = bA64, bvA128
    c["bB64"], c["bvB128"] = bB64, bvB128
    c["bmix32"] = bmix32

    # mixture masks: select the true sq/sp rows, write output col
    # ROFF+(r%18), 0.2 mixture weight baked in.  mkSq covers the packed
    # squares tile M23 (sqA rows 0:54, sqB rows 64:100).
    mkSq = np.zeros((H, 32), np.float32)
    for r in range(54):
        mkSq[r, r % O] = 0.2
    for r in range(64, 100):
        mkSq[r, (r - 64) % O] = 0.2
    mkSpA = np.zeros((H, 32), np.float32)
    for v in range(54):
        mkSpA[va_row(v), v % O] = 0.2
    mkSpB = np.zeros((H, 32), np.float32)
    for r in range(54, 90):
        mkSpB[r, (r - 54) % O] = 0.2
    c["mkSq"] = mkSq.astype(bf)
    c["mkSpA"] = mkSpA.astype(bf)
    c["mkSpB"] = mkSpB.astype(bf)

    def pad128(a):
        out = np.zeros((H, a.shape[1]), a.dtype)
        out[:a.shape[0], :] = a
        return out

    packed = {}
    packed["wpack16"] = np.concatenate([c[n] for n in PACK16], axis=1)
    packed["wpackf"] = np.concatenate(
        [pad128(np.asarray(c[n], np.float32)) for n in PACKF], axis=1)
    return packed


def kernel(x, W1, b1, W2, b2, W3, b3):
    global LAST_RESULTS
    if "nc" not in _cache:
        _cache["nc"] = _build()
    nc = _cache["nc"]

    consts = _prep_consts(np.asarray(W1), np.asarray(b1), np.asarray(W2),
                          np.asarray(b2), np.asarray(W3), np.asarray(b3))
    xT = np.asarray(x).T.astype(ml_dtypes.bfloat16)  # [128, B]

    in_maps = []
    for cix in range(NCORES):
        m = dict(consts)
        m["xT"] = np.ascontiguousarray(xT[:, cix * BSHARD:(cix + 1) * BSHARD])
        in_maps.append(m)

    trace = os.environ.get("KERNEL_TRACE", "0") == "1"
    res = run_bass_kernel_spmd(nc, in_maps, list(range(NCORES)), trace=trace)
    LAST_RESULTS = res

    mean = np.concatenate(
        [r["meanT"].astype(np.float32) for r in res.results], axis=1).T
    var = np.concatenate(
        [r["varT"].astype(np.float32) for r in res.results], axis=1).T
    return (np.ascontiguousarray(mean), np.ascontiguousarray(var))


# revision 27
# speedup vs baseline: 1.8245x; 1.1564x over previous
"""GaussianMixtureMLP Trainium2 kernel (v3.2).

5-expert MLP mixture (128->128->128->2*18) over batch 65536, returning the
per-sample mixture mean and variance [65536, 18].

Data-parallel over batch across 8 NeuronCores (no collectives -- the
mixture reduction is over experts, which stay core-local).

Per 512-column tile (16 tiles/core), per expert m:
  h1 = relu(W1[m] @ x + b1)   PE matmul + ACT/DVE relu (bf16 out)
  h2 = relu(W2[m] @ h1 + b2)  PE matmul + ACT/DVE relu
  W3 combined:                ONE matmul per expert into packed PSUM.

PSUM packing (all ALU reads start at 32-aligned partitions; junk rows in
the over-read windows are zeroed by the mask matmuls downstream):
  psA [128,512] (experts 0-2): means 0:54 | 0.2*mean mix-acc 54:72 |
                               vars 72:126 | zeros
  psB [128,512] (experts 3-4): means 0:36 | zeros | mix-acc 54:72 |
                               vars 72:108 | zeros
Post-processing per tile (softplus = ln(1+exp), no softplus table):
  sqX   = ACT Square(psX[0:64] + b3mean)        -> M[0:64]
  spX   = ACT Exp(psX[64:128] + b3var) ACT Ln   -> M[64:128]
  mean  = DVE stt(psA[32:96] + bmix + psB[32:96]) -> mean4 64-row block
          (true rows at block offset 22:40, matching the mask layout)
  2 mask matmuls (0.2 baked in) -> psSv4 64-row block, cols 22:40
Finals per 2-tile group as [128,512] ops (free-dim-only engine cost):
  m2g = mean4^2 (DVE bf16 2x), vg = psSv4+1e-6-m2g (DVE),
  vfg = relu(vg)+1e-6 (Pool).  DMA out rows 22:40 / 86:104.
"""

import os
import numpy as np
import ml_dtypes

from concourse import bacc, bass, mybir, tile
from concourse.bass_utils import run_bass_kernel_spmd

AF = mybir.ActivationFunctionType
ALU = mybir.AluOpType
F32 = mybir.dt.float32
BF16 = mybir.dt.bfloat16

NCORES = 8
BATCH = 65536
BSHARD = BATCH // NCORES          # 8192
TB = 512                          # free-dim tile (fp32 PSUM bank limit)
NT = BSHARD // TB                 # 16 tiles per core
GR = 3                            # tiles per group (32-row blocks at 0/32/64)
NM = 5
H = 128
O = 18
ROFF = 0                          # true-row offset inside each 32-row block

_cache = {}
LAST_RESULTS = None


def _build():
    nc = bacc.Bacc("TRN2", target_bir_lowering=False, debug=False)

    # all weights travel as two packed tensors (two DMAs at startup
    # instead of thirteen serialized ones)
    W16 = {"w1cat": NM * H, "w2cat": NM * H, "w3A": 3 * H, "w3B": 2 * H,
           "w3mixB": 64, "mkSqA": 32, "mkSqB": 32, "mkSp": 32}
    WF32 = {"b1cat": NM, "b2cat": NM, "bqA128": 1, "bvA64": 1, "bB64": 1,
            "bvB64": 1, "bmix32": 1}
    din = {}
    for name, shape, dt_ in [
        ("xT", [H, BSHARD], BF16),
        ("wpack16", [H, sum(W16.values())], BF16),
        ("wpackf", [H, sum(WF32.values())], F32),
    ]:
        din[name] = nc.dram_tensor(name, shape, dt_, kind="ExternalInput").ap()
    meanT = nc.dram_tensor("meanT", [O, BSHARD], BF16, kind="ExternalOutput").ap()
    varT = nc.dram_tensor("varT", [O, BSHARD], BF16, kind="ExternalOutput").ap()

    with tile.TileContext(nc) as tc:
        with (
            tc.tile_pool(name="w", bufs=1) as wp,
            tc.tile_pool(name="x", bufs=3) as xp,
            tc.tile_pool(name="h", bufs=6) as hp,
            tc.tile_pool(name="m", bufs=4) as mp,
            tc.tile_pool(name="g", bufs=2) as gp,
            tc.tile_pool(name="e", bufs=6) as ep,
            tc.tile_pool(name="v", bufs=6) as vp,
            tc.tile_pool(name="l", bufs=2) as lp,
            tc.tile_pool(name="ps1", bufs=2, space="PSUM") as ph1,
            tc.tile_pool(name="ps2", bufs=3, space="PSUM") as ph2,
            tc.tile_pool(name="psa", bufs=1, space="PSUM") as pA,
            tc.tile_pool(name="psb", bufs=1, space="PSUM") as pB,
            tc.tile_pool(name="pss", bufs=1, space="PSUM") as pS,
        ):
            def xdma(t):
                xt = xp.tile([H, TB], BF16, tag="xt")
                nc.sync.dma_start(out=xt, in_=din["xT"][:, t * TB:(t + 1) * TB])
                return xt

            xts = {0: xdma(0)}
            w16 = wp.tile([H, sum(W16.values())], BF16, tag="w16")
            nc.sync.dma_start(out=w16, in_=din["wpack16"])
            wf = wp.tile([H, sum(WF32.values())], F32, tag="wf")
            nc.sync.dma_start(out=wf, in_=din["wpackf"])
            xts[1] = xdma(1)

            w = {}
            off = 0
            for name, width in W16.items():
                w[name] = w16[:, off:off + width]
                off += width
            off = 0
            for name, width in WF32.items():
                w[name] = wf[:, off:off + width]
                off += width
            w["bvA64"] = w["bvA64"][0:64, :]
            w["bB64"] = w["bB64"][0:64, :]
            w["bvB64"] = w["bvB64"][0:64, :]
            w["bmix32"] = w["bmix32"][0:32, :]

            psSv4 = None
            mean4 = None
            EABg = None
            gVG1 = []
            gEAB = []
            for t in range(NT):
                g, k = t // GR, t % GR
                if t + 2 < NT:
                    xts[t + 2] = xdma(t + 2)
                xt = xts.pop(t)

                gsize = min(GR, NT - g * GR)
                rows = 32 * gsize
                if k == 0:
                    psSv4 = pS.tile([rows, TB], F32, tag="psSv4")
                    mean4 = gp.tile([rows, TB], BF16, tag="mean4")
                    EABg = ep.tile([H, gsize * TB], BF16, tag="EAB")

                psA = pA.tile([H, TB], F32, tag="psA")
                psB = pB.tile([H, TB], F32, tag="psB")

                # software-pipelined emission: W2(m) trails W1(m+2) and
                # W3(m) trails further, so parked matmuls (waiting on a
                # relu) never head-of-line-block the PE queue.
                h1s, h2s = {}, {}

                def emit_w1(m):
                    phA = ph1.tile([H, TB], F32, tag="psH1")
                    nc.tensor.matmul(phA, w["w1cat"][:, m * H:(m + 1) * H],
                                     xt, start=True, stop=True)
                    h1 = hp.tile([H, TB], BF16, tag="h1")
                    if m < 2:
                        nc.scalar.activation(h1, phA, AF.Relu,
                                             bias=w["b1cat"][:, m:m + 1])
                    else:
                        nc.vector.tensor_scalar(h1, phA, w["b1cat"][:, m:m + 1],
                                                0.0, ALU.add, ALU.max)
                    h1s[m] = h1

                def emit_w2(m):
                    phB = ph2.tile([H, TB], F32, tag="psH2")
                    nc.tensor.matmul(phB, w["w2cat"][:, m * H:(m + 1) * H],
                                     h1s[m], start=True, stop=True)
                    h2 = hp.tile([H, TB], BF16, tag="h2")
                    if m == 0:
                        nc.scalar.activation(h2, phB, AF.Relu,
                                             bias=w["b2cat"][:, m:m + 1])
                    else:
                        nc.vector.tensor_scalar(h2, phB, w["b2cat"][:, m:m + 1],
                                                0.0, ALU.add, ALU.max)
                    h2s[m] = h2

                def emit_w3(m):
                    h2 = h2s[m]
                    if m < 3:
                        nc.tensor.matmul(psA, w["w3A"][:, m * H:(m + 1) * H],
                                         h2, start=(m == 0), stop=False,
                                         skip_group_check=True)
                    else:
                        nc.tensor.matmul(psB, w["w3B"][:, (m - 3) * H:(m - 2) * H],
                                         h2, start=(m == 3), stop=(m == 4),
                                         skip_group_check=True)
                        # expert 3/4 mixture-mean contribution into psA's
                        # mix rows (psA is the single-PSUM mean source)
                        nc.tensor.matmul(psA[0:32, :],
                                         w["w3mixB"][:, (m - 3) * 32:(m - 2) * 32],
                                         h2, start=False, stop=(m == 4),
                                         skip_group_check=True)

                for m in range(NM):
                    emit_w1(m)
                    if m >= 2:
                        emit_w2(m - 2)
                    if m >= 4:
                        emit_w3(m - 4)
                for m in range(3, NM):
                    emit_w2(m)
                    emit_w3(m - 3)
                for m in range(2, NM):
                    emit_w3(m)

                SQA = mp.tile([H, TB], BF16, tag="SQA")
                MB = mp.tile([64, TB], BF16, tag="MB")
                # full-tile square (junk rows masked later); exps of A and B
                # pack into the top/bottom partition halves of one EAB column
                # slice, so the deferred tail-ln processes half the columns.
                # ln runs post-loop so its ACT table loads twice per kernel.
                nc.scalar.activation(SQA, psA, AF.Square, bias=w["bqA128"])
                nc.scalar.activation(MB, psB[0:64, :], AF.Square,
                                     bias=w["bB64"])
                nc.scalar.activation(EABg[0:64, k * TB:(k + 1) * TB],
                                     psA[64:128, :], AF.Exp, bias=w["bvA64"])
                nc.scalar.activation(EABg[64:128, k * TB:(k + 1) * TB],
                                     psB[64:128, :], AF.Exp, bias=w["bvB64"])

                blk = psSv4[32 * k:32 * (k + 1), :]
                nc.tensor.matmul(blk, w["mkSqA"], SQA, start=True, stop=False,
                                 skip_group_check=True)
                nc.tensor.matmul(blk, w["mkSqB"][0:64, :], MB,
                                 start=False, stop=True,
                                 skip_group_check=True)

                nc.vector.tensor_scalar(mean4[32 * k:32 * (k + 1), :],
                                        psA[0:32, :], w["bmix32"],
                                        None, ALU.add)

                nc.sync.dma_start(
                    out=meanT[:, t * TB:(t + 1) * TB],
                    in_=mean4[32 * k + ROFF:32 * k + ROFF + O, :])

                if k == gsize - 1:
                    m2g = gp.tile([rows, TB], BF16, tag="m2g")
                    nc.gpsimd.tensor_tensor(m2g, mean4, mean4, ALU.mult)
                    vg1 = vp.tile([rows, TB], F32, tag="vg1")
                    nc.vector.scalar_tensor_tensor(vg1, psSv4, 1e-6, m2g,
                                                   ALU.add, ALU.subtract)
                    gVG1.append(vg1)
                    gEAB.append((EABg, gsize))

            # ---- tail: softplus ln + sp mixture sums + variance finals
            # (negative-priority so the scheduler keeps every ln after the
            # main loop's exp/relu/square ACT ops: one ln-table load) ----
            ctx_tail = tc.high_priority(offset=-10**6)
            ctx_tail.__enter__()
            for g, ((EABt, gsize), vg1) in enumerate(zip(gEAB, gVG1)):
                rows = 32 * gsize
                Lg = lp.tile([H, gsize * TB], BF16, tag="Lg")
                nc.scalar.activation(Lg, EABt, AF.Ln, bias=1.0)
                psSvT = pS.tile([rows, TB], F32, tag="psSv4")
                for kk in range(gsize):
                    blk = psSvT[32 * kk:32 * (kk + 1), :]
                    nc.tensor.matmul(blk, w["mkSp"],
                                     Lg[:, kk * TB:(kk + 1) * TB],
                                     start=True, stop=True,
                                     skip_group_check=True)
                vgf = gp.tile([rows, TB], F32, tag="vgf")
                nc.vector.tensor_tensor(vgf, psSvT, vg1, ALU.add)
                vfg = gp.tile([rows, TB], BF16, tag="vfg")
                nc.gpsimd.tensor_scalar(vfg, vgf, 0.0, 1e-6,
                                        ALU.max, ALU.add)
                for kk in range(gsize):
                    tt = g * GR + kk
                    nc.sync.dma_start(
                        out=varT[:, tt * TB:(tt + 1) * TB],
                        in_=vfg[32 * kk + ROFF:32 * kk + ROFF + O, :])
            ctx_tail.__exit__(None, None, None)
    nc.compile()
    return nc


def _prep_consts(W1, b1, W2, b2, W3, b3):
    bf = ml_dtypes.bfloat16
    c = {}
    PACK16 = ["w1cat", "w2cat", "w3A", "w3B", "w3mixB", "mkSqA", "mkSqB",
              "mkSp"]
    PACKF = ["b1cat", "b2cat", "bqA128", "bvA64", "bB64", "bvB64", "bmix32"]
    c["w1cat"] = np.concatenate([W1[m].T for m in range(NM)], axis=1).astype(bf)
    c["w2cat"] = np.concatenate([W2[m].T for m in range(NM)], axis=1).astype(bf)

    # psA rows: mix accumulator 0:18 | meansA 18:72 | varsA 72:126.
    # psB rows: meansB 0:36 | varsB 64:100.  Experts 3/4 add their mix
    # part via [H,32] matmuls into psA[0:32].
    def w3blk(m):
        blk = np.zeros((H, H), np.float32)
        if m < 3:
            blk[:, 0:O] = 0.2 * W3[m, 0:O, :].T
            blk[:, 18 + m * O:18 + (m + 1) * O] = W3[m, 0:O, :].T
            blk[:, 72 + m * O:72 + (m + 1) * O] = W3[m, O:2 * O, :].T
        else:
            j = m - 3
            blk[:, j * O:(j + 1) * O] = W3[m, 0:O, :].T
            blk[:, 64 + j * O:64 + (j + 1) * O] = W3[m, O:2 * O, :].T
        return blk

    c["w3A"] = np.concatenate([w3blk(m) for m in range(3)], axis=1).astype(bf)
    c["w3B"] = np.concatenate([w3blk(m) for m in (3, 4)], axis=1).astype(bf)
    w3mixB = np.zeros((H, 64), np.float32)
    for j, m in enumerate((3, 4)):
        w3mixB[:, 32 * j:32 * j + O] = 0.2 * W3[m, 0:O, :].T
    c["w3mixB"] = w3mixB.astype(bf)

    c["b1cat"] = np.ascontiguousarray(b1.T, np.float32)
    c["b2cat"] = np.ascontiguousarray(b2.T, np.float32)

    bqA128 = np.zeros((H, 1), np.float32)
    bqA128[18:72, 0] = np.concatenate([b3[m, 0:O] for m in range(3)])
    bvA64 = np.zeros((64, 1), np.float32)
    bvA64[8:62, 0] = np.concatenate([b3[m, O:2 * O] for m in range(3)])
    bB64 = np.zeros((64, 1), np.float32)
    bB64[0:36, 0] = np.concatenate([b3[m, 0:O] for m in (3, 4)])
    bvB64 = np.zeros((64, 1), np.float32)
    bvB64[0:36, 0] = np.concatenate([b3[m, O:2 * O] for m in (3, 4)])
    bmix32 = np.zeros((32, 1), np.float32)
    bmix32[0:O, 0] = 0.2 * b3[:, 0:O].sum(axis=0)
    c["bqA128"], c["bvA64"] = bqA128, bvA64
    c["bB64"], c["bvB64"] = bB64, bvB64
    c["bmix32"] = bmix32

    # mixture masks: select true sq/sp rows, write col r%18 with the 0.2
    # mixture weight baked in.  mkSpB lives in partitions 64:128 so its
    # base partition matches the bottom-half ln rows it consumes.
    mkSqA = np.zeros((H, 32), np.float32)
    for r in range(18, 72):
        mkSqA[r, (r - 18) % O] = 0.2
    mkSqB = np.zeros((H, 32), np.float32)
    for r in range(36):
        mkSqB[r, r % O] = 0.2
    mkSp = np.zeros((H, 32), np.float32)
    for v in range(54):
        mkSp[8 + v, v % O] = 0.2        # A-side softplus rows (top half)
    for r in range(36):
        mkSp[64 + r, r % O] = 0.2       # B-side softplus rows (bottom half)
    c["mkSqA"] = mkSqA.astype(bf)
    c["mkSqB"] = mkSqB.astype(bf)
    c["mkSp"] = mkSp.astype(bf)

    def pad128(a):
        out = np.zeros((H, a.shape[1]), a.dtype)
        out[:a.shape[0], :] = a
        return out

    packed = {}
    packed["wpack16"] = np.concatenate([c[n] for n in PACK16], axis=1)
    packed["wpackf"] = np.concatenate(
        [pad128(np.asarray(c[n], np.float32)) for n in PACKF], axis=1)
    return packed


def kernel(x, W1, b1, W2, b2, W3, b3):
    global LAST_RESULTS
    if "nc" not in _cache:
        _cache["nc"] = _build()
    nc = _cache["nc"]

    consts = _prep_consts(np.asarray(W1), np.asarray(b1), np.asarray(W2),
                          np.asarray(b2), np.asarray(W3), np.asarray(b3))
    xT = np.asarray(x).T.astype(ml_dtypes.bfloat16)  # [128, B]

    in_maps = []
    for cix in range(NCORES):
        m = dict(consts)
        m["xT"] = np.ascontiguousarray(xT[:, cix * BSHARD:(cix + 1) * BSHARD])
        in_maps.append(m)

    trace = os.environ.get("KERNEL_TRACE", "0") == "1"
    res = run_bass_kernel_spmd(nc, in_maps, list(range(NCORES)), trace=trace)
    LAST_RESULTS = res

    mean = np.concatenate(
        [r["meanT"].astype(np.float32) for r in res.results], axis=1).T
    var = np.concatenate(
        [r["varT"].astype(np.float32) for r in res.results], axis=1).T
    return (np.ascontiguousarray(mean), np.ascontiguousarray(var))


# revision 38
# speedup vs baseline: 1.8922x; 1.0371x over previous
"""GaussianMixtureMLP Trainium2 kernel (v3.2).

5-expert MLP mixture (128->128->128->2*18) over batch 65536, returning the
per-sample mixture mean and variance [65536, 18].

Data-parallel over batch across 8 NeuronCores (no collectives -- the
mixture reduction is over experts, which stay core-local).

Per 512-column tile (16 tiles/core), per expert m:
  h1 = relu(W1[m] @ x + b1)   PE matmul + ACT/DVE relu (bf16 out)
  h2 = relu(W2[m] @ h1 + b2)  PE matmul + ACT/DVE relu
  W3 combined:                ONE matmul per expert into packed PSUM.

PSUM packing (all ALU reads start at 32-aligned partitions; junk rows in
the over-read windows are zeroed by the mask matmuls downstream):
  psA [128,512] (experts 0-2): means 0:54 | 0.2*mean mix-acc 54:72 |
                               vars 72:126 | zeros
  psB [128,512] (experts 3-4): means 0:36 | zeros | mix-acc 54:72 |
                               vars 72:108 | zeros
Post-processing per tile (softplus = ln(1+exp), no softplus table):
  sqX   = ACT Square(psX[0:64] + b3mean)        -> M[0:64]
  spX   = ACT Exp(psX[64:128] + b3var) ACT Ln   -> M[64:128]
  mean  = DVE stt(psA[32:96] + bmix + psB[32:96]) -> mean4 64-row block
          (true rows at block offset 22:40, matching the mask layout)
  2 mask matmuls (0.2 baked in) -> psSv4 64-row block, cols 22:40
Finals per 2-tile group as [128,512] ops (free-dim-only engine cost):
  m2g = mean4^2 (DVE bf16 2x), vg = psSv4+1e-6-m2g (DVE),
  vfg = relu(vg)+1e-6 (Pool).  DMA out rows 22:40 / 86:104.
"""

import os
import numpy as np
import ml_dtypes

from concourse import bacc, bass, mybir, tile
from concourse.bass_utils import run_bass_kernel_spmd

AF = mybir.ActivationFunctionType
ALU = mybir.AluOpType
F32 = mybir.dt.float32
BF16 = mybir.dt.bfloat16

NCORES = 8
BATCH = 65536
BSHARD = BATCH // NCORES          # 8192
TB = 512                          # free-dim tile (fp32 PSUM bank limit)
NT = BSHARD // TB                 # 16 tiles per core
GR = 3                            # tiles per group (32-row blocks at 0/32/64)
NM = 5
H = 128
O = 18
ROFF = 0                          # true-row offset inside each 32-row block

_cache = {}
LAST_RESULTS = None


def _build():
    nc = bacc.Bacc("TRN2", target_bir_lowering=False, debug=False)

    # all weights travel as two packed tensors (two DMAs at startup
    # instead of thirteen serialized ones)
    W16A = {"w1cat": NM * H, "w2cat": NM * H}
    W16B = {"w3A": 3 * H, "w3B": 2 * H, "w3mixB": 64, "mkSqA": 32,
            "mkSqB": 32, "mkSp": 32}
    WF32 = {"b1cat": NM, "b2cat": NM, "bqA128": 1, "bvA64": 1, "bB64": 1,
            "bvB64": 1, "bmix32": 1}
    din = {}
    for name, shape, dt_ in [
        ("xT", [H, BSHARD], BF16),
        ("wpack16a", [H, sum(W16A.values())], BF16),
        ("wpack16b", [H, sum(W16B.values())], BF16),
        ("wpackf", [H, sum(WF32.values())], F32),
    ]:
        din[name] = nc.dram_tensor(name, shape, dt_, kind="ExternalInput").ap()
    meanT = nc.dram_tensor("meanT", [O, BSHARD], BF16, kind="ExternalOutput").ap()
    varT = nc.dram_tensor("varT", [O, BSHARD], BF16, kind="ExternalOutput").ap()

    with tile.TileContext(nc) as tc:
        with (
            tc.tile_pool(name="w", bufs=1) as wp,
            tc.tile_pool(name="x", bufs=3) as xp,
            tc.tile_pool(name="h", bufs=6) as hp,
            tc.tile_pool(name="m", bufs=4) as mp,
            tc.tile_pool(name="g", bufs=2) as gp,
            tc.tile_pool(name="e", bufs=6) as ep,
            tc.tile_pool(name="v", bufs=6) as vp,
            tc.tile_pool(name="l", bufs=2) as lp,
            tc.tile_pool(name="ps1", bufs=2, space="PSUM") as ph1,
            tc.tile_pool(name="ps2", bufs=2, space="PSUM") as ph2,
            tc.tile_pool(name="psa", bufs=1, space="PSUM") as pA,
            tc.tile_pool(name="psb", bufs=1, space="PSUM") as pB,
            tc.tile_pool(name="pss", bufs=2, space="PSUM") as pS,
        ):
            def xdma(t):
                xt = xp.tile([H, TB], BF16, tag="xt")
                nc.sync.dma_start(out=xt, in_=din["xT"][:, t * TB:(t + 1) * TB])
                return xt

            warm = wp.tile([32, 1], F32, tag="warm")
            nc.vector.memset(warm, 0.0)
            nc.scalar.activation(warm, warm, AF.Relu)

            xts = {0: xdma(0)}
            w16a = wp.tile([H, sum(W16A.values())], BF16, tag="w16a")
            nc.sync.dma_start(out=w16a, in_=din["wpack16a"])
            wf = wp.tile([H, sum(WF32.values())], F32, tag="wf")
            nc.sync.dma_start(out=wf, in_=din["wpackf"])
            w16b = wp.tile([H, sum(W16B.values())], BF16, tag="w16b")
            nc.sync.dma_start(out=w16b, in_=din["wpack16b"])
            xts[1] = xdma(1)

            w = {}
            off = 0
            for name, width in W16A.items():
                w[name] = w16a[:, off:off + width]
                off += width
            off = 0
            for name, width in W16B.items():
                w[name] = w16b[:, off:off + width]
                off += width
            off = 0
            for name, width in WF32.items():
                w[name] = wf[:, off:off + width]
                off += width
            w["bvA64"] = w["bvA64"][0:64, :]
            w["bB64"] = w["bB64"][0:64, :]
            w["bvB64"] = w["bvB64"][0:64, :]
            w["bmix32"] = w["bmix32"][0:32, :]

            psSv4 = None
            mean4 = None
            EABg = None
            pending_finals = None
            gVG1 = []
            gEAB = []
            for t in range(NT):
                g, k = t // GR, t % GR
                if t + 2 < NT:
                    xts[t + 2] = xdma(t + 2)
                xt = xts.pop(t)

                gsize = min(GR, NT - g * GR)
                rows = 32 * gsize
                if k == 0:
                    psSv4 = pS.tile([rows, TB], F32, tag="psSv4")
                    mean4 = gp.tile([rows, TB], BF16, tag="mean4")
                    EABg = ep.tile([H, gsize * TB], BF16, tag="EAB")

                psA = pA.tile([H, TB], F32, tag="psA")
                psB = pB.tile([H, TB], F32, tag="psB")

                # software-pipelined emission: W2(m) trails W1(m+2) and
                # W3(m) trails further, so parked matmuls (waiting on a
                # relu) never head-of-line-block the PE queue.
                h1s, h2s = {}, {}

                def emit_w1(m):
                    phA = ph1.tile([H, TB], F32, tag="psH1")
                    nc.tensor.matmul(phA, w["w1cat"][:, m * H:(m + 1) * H],
                                     xt, start=True, stop=True)
                    h1 = hp.tile([H, TB], BF16, tag="h1")
                    if m < 2:
                        nc.scalar.activation(h1, phA, AF.Relu,
                                             bias=w["b1cat"][:, m:m + 1])
                    else:
                        nc.vector.tensor_scalar(h1, phA, w["b1cat"][:, m:m + 1],
                                                0.0, ALU.add, ALU.max)
                    h1s[m] = h1

                def emit_w2(m):
                    phB = ph2.tile([H, TB], F32, tag="psH2")
                    nc.tensor.matmul(phB, w["w2cat"][:, m * H:(m + 1) * H],
                                     h1s[m], start=True, stop=True)
                    h2 = hp.tile([H, TB], BF16, tag="h2")
                    if m == 0:
                        nc.scalar.activation(h2, phB, AF.Relu,
                                             bias=w["b2cat"][:, m:m + 1])
                    else:
                        nc.vector.tensor_scalar(h2, phB, w["b2cat"][:, m:m + 1],
                                                0.0, ALU.add, ALU.max)
                    h2s[m] = h2

                def emit_w3(m):
                    h2 = h2s[m]
                    if m < 3:
                        nc.tensor.matmul(psA, w["w3A"][:, m * H:(m + 1) * H],
                                         h2, start=(m == 0), stop=False,
                                         skip_group_check=True)
                    else:
                        nc.tensor.matmul(psB, w["w3B"][:, (m - 3) * H:(m - 2) * H],
                                         h2, start=(m == 3), stop=(m == 4),
                                         skip_group_check=True)
                        # expert 3/4 mixture-mean contribution into psA's
                        # mix rows (psA is the single-PSUM mean source)
                        nc.tensor.matmul(psA[0:32, :],
                                         w["w3mixB"][:, (m - 3) * 32:(m - 2) * 32],
                                         h2, start=False, stop=(m == 4),
                                         skip_group_check=True)

                for m in range(NM):
                    emit_w1(m)
                    if m == 1 and pending_finals is not None:
                        pending_finals()
                        pending_finals = None
                    if m >= 2:
                        emit_w2(m - 2)
                    if m >= 4:
                        emit_w3(m - 4)
                for m in range(3, NM):
                    emit_w2(m)
                    emit_w3(m - 3)
                for m in range(2, NM):
                    emit_w3(m)

                SQA = mp.tile([H, TB], BF16, tag="SQA")
                MB = mp.tile([64, TB], BF16, tag="MB")
                # full-tile square (junk rows masked later); exps of A and B
                # pack into the top/bottom partition halves of one EAB column
                # slice, so the deferred tail-ln processes half the columns.
                # ln runs post-loop so its ACT table loads twice per kernel.
                nc.scalar.activation(SQA, psA, AF.Square, bias=w["bqA128"])
                nc.scalar.activation(MB, psB[0:64, :], AF.Square,
                                     bias=w["bB64"])
                nc.scalar.activation(EABg[0:64, k * TB:(k + 1) * TB],
                                     psA[64:128, :], AF.Exp, bias=w["bvA64"])
                nc.scalar.activation(EABg[64:128, k * TB:(k + 1) * TB],
                                     psB[64:128, :], AF.Exp, bias=w["bvB64"])

                blk = psSv4[32 * k:32 * (k + 1), :]
                nc.tensor.matmul(blk, w["mkSqA"], SQA, start=True, stop=False,
                                 skip_group_check=True)
                nc.tensor.matmul(blk, w["mkSqB"][0:64, :], MB,
                                 start=False, stop=True,
                                 skip_group_check=True)

                nc.vector.tensor_scalar(mean4[32 * k:32 * (k + 1), :],
                                        psA[0:32, :], w["bmix32"],
                                        None, ALU.add)

                nc.sync.dma_start(
                    out=meanT[:, t * TB:(t + 1) * TB],
                    in_=mean4[32 * k + ROFF:32 * k + ROFF + O, :])

                if k == gsize - 1:
                    def mk_finals(rows=rows, mean4=mean4, psSv4=psSv4,
                                  EABg=EABg, gsize=gsize):
                        m2g = gp.tile([rows, TB], BF16, tag="m2g")
                        nc.vector.tensor_tensor(m2g, mean4, mean4, ALU.mult)
                        vg1 = vp.tile([rows, TB], F32, tag="vg1")
                        nc.vector.scalar_tensor_tensor(vg1, psSv4, 1e-6, m2g,
                                                       ALU.add, ALU.subtract)
                        gVG1.append(vg1)
                        gEAB.append((EABg, gsize))
                    if t == NT - 1:
                        mk_finals()
                    else:
                        pending_finals = mk_finals

            # ---- tail: softplus ln + sp mixture sums + variance finals
            # (negative-priority so the scheduler keeps every ln after the
            # main loop's exp/relu/square ACT ops: one ln-table load) ----
            ctx_tail = tc.high_priority(offset=-10**6)
            ctx_tail.__enter__()
            for g, ((EABt, gsize), vg1) in enumerate(zip(gEAB, gVG1)):
                rows = 32 * gsize
                Lg = lp.tile([H, gsize * TB], BF16, tag="Lg")
                nc.scalar.activation(Lg, EABt, AF.Ln, bias=1.0)
                psSvT = pS.tile([rows, TB], F32, tag="psSv4")
                for kk in range(gsize):
                    blk = psSvT[32 * kk:32 * (kk + 1), :]
                    nc.tensor.matmul(blk, w["mkSp"],
                                     Lg[:, kk * TB:(kk + 1) * TB],
                                     start=True, stop=True,
                                     skip_group_check=True)
                vgf = gp.tile([rows, TB], F32, tag="vgf")
                nc.vector.tensor_tensor(vgf, psSvT, vg1, ALU.add)
                vfg = gp.tile([rows, TB], BF16, tag="vfg")
                nc.gpsimd.tensor_scalar(vfg, vgf, 0.0, 1e-6,
                                        ALU.max, ALU.add)
                for kk in range(gsize):
                    tt = g * GR + kk
                    nc.sync.dma_start(
                        out=varT[:, tt * TB:(tt + 1) * TB],
                        in_=vfg[32 * kk + ROFF:32 * kk + ROFF + O, :])
            ctx_tail.__exit__(None, None, None)
    nc.compile()
    return nc


def _prep_consts(W1, b1, W2, b2, W3, b3):
    bf = ml_dtypes.bfloat16
    c = {}
    PACK16A = ["w1cat", "w2cat"]
    PACK16B = ["w3A", "w3B", "w3mixB", "mkSqA", "mkSqB", "mkSp"]
    PACKF = ["b1cat", "b2cat", "bqA128", "bvA64", "bB64", "bvB64", "bmix32"]
    c["w1cat"] = np.concatenate([W1[m].T for m in range(NM)], axis=1).astype(bf)
    c["w2cat"] = np.concatenate([W2[m].T for m in range(NM)], axis=1).astype(bf)

    # psA rows: mix accumulator 0:18 | meansA 18:72 | varsA 72:126.
    # psB rows: meansB 0:36 | varsB 64:100.  Experts 3/4 add their mix
    # part via [H,32] matmuls into psA[0:32].
    def w3blk(m):
        blk = np.zeros((H, H), np.float32)
        if m < 3:
            blk[:, 0:O] = 0.2 * W3[m, 0:O, :].T
            blk[:, 18 + m * O:18 + (m + 1) * O] = W3[m, 0:O, :].T
            blk[:, 72 + m * O:72 + (m + 1) * O] = W3[m, O:2 * O, :].T
        else:
            j = m - 3
            blk[:, j * O:(j + 1) * O] = W3[m, 0:O, :].T
            blk[:, 64 + j * O:64 + (j + 1) * O] = W3[m, O:2 * O, :].T
        return blk

    c["w3A"] = np.concatenate([w3blk(m) for m in range(3)], axis=1).astype(bf)
    c["w3B"] = np.concatenate([w3blk(m) for m in (3, 4)], axis=1).astype(bf)
    w3mixB = np.zeros((H, 64), np.float32)
    for j, m in enumerate((3, 4)):
        w3mixB[:, 32 * j:32 * j + O] = 0.2 * W3[m, 0:O, :].T
    c["w3mixB"] = w3mixB.astype(bf)

    c["b1cat"] = np.ascontiguousarray(b1.T, np.float32)
    c["b2cat"] = np.ascontiguousarray(b2.T, np.float32)

    bqA128 = np.zeros((H, 1), np.float32)
    bqA128[18:72, 0] = np.concatenate([b3[m, 0:O] for m in range(3)])
    bvA64 = np.zeros((64, 1), np.float32)
    bvA64[8:62, 0] = np.concatenate([b3[m, O:2 * O] for m in range(3)])
    bB64 = np.zeros((64, 1), np.float32)
    bB64[0:36, 0] = np.concatenate([b3[m, 0:O] for m in (3, 4)])
    bvB64 = np.zeros((64, 1), np.float32)
    bvB64[0:36, 0] = np.concatenate([b3[m, O:2 * O] for m in (3, 4)])
    bmix32 = np.zeros((32, 1), np.float32)
    bmix32[0:O, 0] = 0.2 * b3[:, 0:O].sum(axis=0)
    c["bqA128"], c["bvA64"] = bqA128, bvA64
    c["bB64"], c["bvB64"] = bB64, bvB64
    c["bmix32"] = bmix32

    # mixture masks: select true sq/sp rows, write col r%18 with the 0.2
    # mixture weight baked in.  mkSpB lives in partitions 64:128 so its
    # base partition matches the bottom-half ln rows it consumes.
    mkSqA = np.zeros((H, 32), np.float32)
    for r in range(18, 72):
        mkSqA[r, (r - 18) % O] = 0.2
    mkSqB = np.zeros((H, 32), np.float32)
    for r in range(36):
        mkSqB[r, r % O] = 0.2
    mkSp = np.zeros((H, 32), np.float32)
    for v in range(54):
        mkSp[8 + v, v % O] = 0.2        # A-side softplus rows (top half)
    for r in range(36):
        mkSp[64 + r, r % O] = 0.2       # B-side softplus rows (bottom half)
    c["mkSqA"] = mkSqA.astype(bf)
    c["mkSqB"] = mkSqB.astype(bf)
    c["mkSp"] = mkSp.astype(bf)

    def pad128(a):
        out = np.zeros((H, a.shape[1]), a.dtype)
        out[:a.shape[0], :] = a
        return out

    packed = {}
    packed["wpack16a"] = np.concatenate([c[n] for n in PACK16A], axis=1)
    packed["wpack16b"] = np.concatenate([c[n] for n in PACK16B], axis=1)
    packed["wpackf"] = np.concatenate(
        [pad128(np.asarray(c[n], np.float32)) for n in PACKF], axis=1)
    return packed


def kernel(x, W1, b1, W2, b2, W3, b3):
    global LAST_RESULTS
    if "nc" not in _cache:
        _cache["nc"] = _build()
    nc = _cache["nc"]

    consts = _prep_consts(np.asarray(W1), np.asarray(b1), np.asarray(W2),
                          np.asarray(b2), np.asarray(W3), np.asarray(b3))
    xT = np.asarray(x).T.astype(ml_dtypes.bfloat16)  # [128, B]

    in_maps = []
    for cix in range(NCORES):
        m = dict(consts)
        m["xT"] = np.ascontiguousarray(xT[:, cix * BSHARD:(cix + 1) * BSHARD])
        in_maps.append(m)

    trace = os.environ.get("KERNEL_TRACE", "0") == "1"
    res = run_bass_kernel_spmd(nc, in_maps, list(range(NCORES)), trace=trace)
    LAST_RESULTS = res

    mean = np.concatenate(
        [r["meanT"].astype(np.float32) for r in res.results], axis=1).T
    var = np.concatenate(
        [r["varT"].astype(np.float32) for r in res.results], axis=1).T
    return (np.ascontiguousarray(mean), np.ascontiguousarray(var))
